# revision 14
# baseline (speedup 1.0000x reference)
import sys

if "/opt/trn_rl_repo" not in sys.path:
    sys.path.insert(0, "/opt/trn_rl_repo")

import numpy as np
import ml_dtypes

from concourse import bass, tile, bacc
from concourse.bass import mybir

F32 = mybir.dt.float32
BF16 = mybir.dt.bfloat16

N_CORES = 8
N_TOTAL = 32768
N_CORE = N_TOTAL // N_CORES  # 4096 rows per core
D = 1024
C = 64
K = 16
DEPTH = 4
M = 1024
N_STAGES = 2
ROWS_STAGE = N_CORE // N_STAGES  # 2048
ALU = mybir.AluOpType
AFT = mybir.ActivationFunctionType


def build_program(dims=None, repeat=1, widths0=(512, 512)):
    """dims kept for signature compat (gather now happens on host)."""
    nc = bacc.Bacc()
    xg_d = nc.declare_dram_parameter(
        "xg", [N_STAGES, 128, DEPTH, ROWS_STAGE // 2], F32, isOutput=False
    )
    thr_d = nc.declare_dram_parameter("thrcols", [128, 15], F32, isOutput=False)
    lut_d = nc.declare_dram_parameter("lutT", [C * K, M], BF16, isOutput=False)
    kvec_d = nc.declare_dram_parameter("kvec", [128, 1], F32, isOutput=False)
    out_d = nc.declare_dram_parameter("out", [N_CORE, M], BF16, isOutput=True)

    NH = ROWS_STAGE // 2  # 1024 rows per partition-half

    with tile.TileContext(nc) as tc:
        from contextlib import ExitStack
        es = ExitStack()
        pers = es.enter_context(tc.tile_pool(name="pers", bufs=1))

        def ptile(shape, dtype, name):
            return pers.tile(shape, dtype, name=name, tag=name)

        # ---- persistent tiles ----
        lutT = ptile([128, 8, M], BF16, "lutT_sb")       # [j*128+p row, m]
        thr = ptile([128, 15], F32, "thr_sb")
        kvec = ptile([128, 1], F32, "kvec_sb")
        tmps = [ptile([128, NH], F32, f"tmp{ti}_sb") for ti in range(10)]
        b0, b1, b2, tA, tB, tC, tD, tE, tF, tG = tmps
        I8 = mybir.dt.int8
        b0i = ptile([128, NH], I8, "b0i_sb")
        b1i = ptile([128, NH], I8, "b1i_sb")

        xpool = es.enter_context(tc.tile_pool(name="xpool", bufs=2))
        bpool = es.enter_context(tc.tile_pool(name="bpool", bufs=2))
        epool = es.enter_context(tc.tile_pool(name="epool", bufs=2))
        opool = es.enter_context(tc.tile_pool(name="opool", bufs=2))
        pspool = es.enter_context(
            tc.tile_pool(name="pspool", bufs=2, space=bass.MemorySpace.PSUM)
        )

        nc.sync.dma_start(thr[:], thr_d[:])
        nc.sync.dma_start(kvec[:], kvec_d[:])
        for j in range(8):
            nc.scalar.dma_start(lutT[:, j, :], lut_d[j * 128:(j + 1) * 128, :])

        def tcol(i):
            return thr[:, i:i + 1]

        def descent(chT, bucketbf, fsl):
            """tree descent for n-slice fsl; writes bucket values to bucketbf."""
            xd = [chT[:, d, fsl] for d in range(DEPTH)]
            t = [tt[:, fsl] for tt in tmps]
            b0_, b1_, b2_, tA_, tB_, tC_, tD_, tE_, tF_, tG_ = t
            b0i_, b1i_ = b0i[:, fsl], b1i[:, fsl]
            V = nc.vector
            V.tensor_scalar(b0_, xd[0], tcol(0), None, ALU.is_gt)
            V.tensor_copy(b0i_, b0_)
            V.tensor_scalar(tA_, b0_, tcol(2), tcol(1), ALU.mult, ALU.add)
            V.tensor_tensor(b1_, xd[1], tA_, ALU.is_gt)
            V.tensor_copy(b1i_, b1_)

            V.tensor_scalar(tB_, b1_, tcol(4), tcol(3), ALU.mult, ALU.add)
            V.tensor_scalar(tC_, b1_, tcol(6), tcol(5), ALU.mult, ALU.add)
            V.tensor_copy(tA_, tB_)
            V.copy_predicated(tA_, b0i_, tC_)
            V.tensor_tensor(b2_, xd[2], tA_, ALU.is_gt)

            V.tensor_scalar(tB_, b2_, tcol(8), tcol(7), ALU.mult, ALU.add)
            V.tensor_scalar(tC_, b2_, tcol(10), tcol(9), ALU.mult, ALU.add)
            V.tensor_scalar(tD_, b2_, tcol(12), tcol(11), ALU.mult, ALU.add)
            V.tensor_scalar(tE_, b2_, tcol(14), tcol(13), ALU.mult, ALU.add)
            V.tensor_copy(tF_, tB_)
            V.copy_predicated(tF_, b1i_, tC_)
            V.tensor_copy(tG_, tD_)
            V.copy_predicated(tG_, b1i_, tE_)
            V.tensor_copy(tA_, tF_)
            V.copy_predicated(tA_, b0i_, tG_)
            V.tensor_tensor(tD_, xd[3], tA_, ALU.is_gt)   # b3 -> tD_

            # bucket = ((b0*2+b1)*2+b2)*2+b3
            V.scalar_tensor_tensor(tB_, b0_, 2.0, b1_, ALU.mult, ALU.add)
            V.scalar_tensor_tensor(tC_, tB_, 2.0, b2_, ALU.mult, ALU.add)
            V.scalar_tensor_tensor(bucketbf[:, fsl], tC_, 2.0, tD_, ALU.mult, ALU.add)

        for rep in range(repeat):
            for s in range(N_STAGES):
                # gathered input, already laid out [p=(hp*64+c), d, n];
                # per-plane DMAs so level-0 compares start after plane 0 lands
                chT = xpool.tile([128, DEPTH, NH], F32, name="chT", tag="chT")
                for d in range(DEPTH):
                    nc.sync.dma_start(chT[:, d, :], xg_d[s, :, d, :])
                bucketbf = bpool.tile([128, NH], BF16, name="bucketbf", tag="bucketbf")
                ET = epool.tile([128, 8, ROWS_STAGE], BF16, name="ET", tag="ET")

                # chunk stage 0 finely so first matmuls start early
                first = rep == 0 and s == 0
                widths = list(widths0) if first else [NH]
                f0 = 0
                for width in widths:
                    fsl = slice(f0, f0 + width)
                    descent(chT, bucketbf, fsl)

                    # E^T: replicate bucket row of channel 8j+cl to partitions
                    # p = k*8 + cl (log2 doubling), then compare k = p//8
                    engs = [nc.sync, nc.scalar]
                    for j in range(8):
                        for hp in range(2):
                            esl = slice(hp * NH + f0, hp * NH + f0 + width)
                            engs[(2 * j + hp) % 2].dma_start(
                                ET[0:8, j, esl],
                                bucketbf[hp * 64 + 8 * j:hp * 64 + 8 * j + 8, fsl],
                            )
                    for dbl in range(4):
                        w = 8 << dbl
                        for hp in range(2):
                            esl = slice(hp * NH + f0, hp * NH + f0 + width)
                            (nc.scalar if (dbl + hp) % 2 == 0 else nc.sync).dma_start(
                                ET[w:2 * w, :, esl], ET[0:w, :, esl]
                            )
                    for j in range(8):
                        for hp in range(2):
                            esl = slice(hp * NH + f0, hp * NH + f0 + width)
                            nc.gpsimd.tensor_scalar(
                                ET[:, j, esl], ET[:, j, esl], kvec[:], None,
                                ALU.is_equal,
                            )

                    # ---- matmul + output for the i-tiles this chunk covers ----
                    i_list = [hp * 8 + f0 // 128 + d
                              for hp in range(2) for d in range(width // 128)]
                    for i in i_list:
                        ps = [
                            pspool.tile([128, 512], F32, name=f"ps{mc}", tag=f"ps{mc}")
                            for mc in range(2)
                        ]
                        for j in range(8):
                            lhsT = ET[:, j, i * 128:(i + 1) * 128]
                            for mc in range(2):
                                nc.tensor.matmul(
                                    ps[mc][:], lhsT,
                                    lutT[:, j, mc * 512:(mc + 1) * 512],
                                    start=(j == 0), stop=(j == 7),
                                )
                        osb = opool.tile([128, M], BF16, name="osb", tag="osb")
                        nc.scalar.activation(osb[:, 0:512], ps[0][:], AFT.Copy)
                        nc.vector.tensor_copy(osb[:, 512:1024], ps[1][:])
                        r0 = s * ROWS_STAGE + i * 128
                        (nc.sync if i % 2 == 0 else nc.scalar).dma_start(
                            out_d[r0:r0 + 128, :], osb[:]
                        )
                    f0 += width
        es.close()
    nc.finalize()
    return nc


def _prep_inputs(inputMatrix, dims, thresholds, lut):
    x = np.asarray(inputMatrix, dtype=np.float32)
    dims_a = np.asarray(dims).ravel().astype(np.int64)
    thr = np.asarray(thresholds, dtype=np.float32).reshape(C, K - 1)
    lut = np.asarray(lut, dtype=np.float32)

    # thrcols [128, 15]: t0,t1,d21,t3,d43,t5,d65,t7,d87,t9,d109,t11,d1211,t13,d1413
    tcols = np.empty((C, 15), dtype=np.float32)
    tcols[:, 0] = thr[:, 0]
    pairs = [(1, 2), (3, 4), (5, 6), (7, 8), (9, 10), (11, 12), (13, 14)]
    for idx, (lo, hi) in enumerate(pairs):
        tcols[:, 1 + 2 * idx] = thr[:, lo]
        tcols[:, 2 + 2 * idx] = thr[:, hi] - thr[:, lo]
    thrcols = np.concatenate([tcols, tcols], axis=0)  # [128, 15]

    # lutT row p = j*128 + k*8 + cl  (channel = 8j+cl), col m
    lutT = (
        lut.reshape(M, 8, 8, K)           # [M, j, cl, k]
        .transpose(1, 3, 2, 0)            # [j, k, cl, M]
        .reshape(C * K, M)
        .astype(ml_dtypes.bfloat16)
    )

    kvec = (np.arange(128) // 8).astype(np.float32).reshape(128, 1)

    # per-core gathered input: xg[s][hp*64+c][d][n] =
    #   x[core*4096 + s*2048 + hp*1024 + n, dims[4c+d]]
    NH = ROWS_STAGE // 2
    xgs = []
    for i in range(N_CORES):
        g = x[i * N_CORE:(i + 1) * N_CORE][:, dims_a]        # [4096, 256]
        g = g.reshape(N_STAGES, 2, NH, C, DEPTH)             # [s, hp, n, c, d]
        xg = np.ascontiguousarray(g.transpose(0, 1, 3, 4, 2)).reshape(
            N_STAGES, 128, DEPTH, NH
        )
        xgs.append(xg)
    return xgs, thrcols, lutT, kvec


def make_in_maps(prepped):
    xgs, thrcols, lutT, kvec = prepped
    return [
        {"xg": xgs[i], "thrcols": thrcols, "lutT": lutT, "kvec": kvec}
        for i in range(N_CORES)
    ]


def kernel(inputMatrix, dims, thresholds, lut, selection_matrix=None,
           tree_des_mat=None):
    from concourse.bass_utils import run_bass_kernel_spmd

    in_maps = make_in_maps(_prep_inputs(inputMatrix, dims, thresholds, lut))
    nc = build_program()
    res = run_bass_kernel_spmd(nc, in_maps, list(range(N_CORES)))
    out = np.concatenate(
        [np.asarray(res.results[i]["out"]) for i in range(N_CORES)], axis=0
    )
    return out.astype(np.float32)


# revision 15
# speedup vs baseline: 4.3914x; 4.3914x over previous
import sys

if "/opt/trn_rl_repo" not in sys.path:
    sys.path.insert(0, "/opt/trn_rl_repo")

import numpy as np
import ml_dtypes

from concourse import bass, tile, bacc
from concourse.bass import mybir

F32 = mybir.dt.float32
BF16 = mybir.dt.bfloat16

N_CORES = 8
N_TOTAL = 32768
N_CORE = N_TOTAL // N_CORES  # 4096 rows per core
D = 1024
C = 64
K = 16
DEPTH = 4
M = 1024
N_STAGES = 2
ROWS_STAGE = N_CORE // N_STAGES  # 2048
ALU = mybir.AluOpType
AFT = mybir.ActivationFunctionType


def build_program(dims=None, repeat=1, widths0=(512, 512)):
    """dims kept for signature compat (gather now happens on host)."""
    nc = bacc.Bacc()
    xg_d = nc.declare_dram_parameter(
        "xg", [N_STAGES, 128, DEPTH, ROWS_STAGE // 2], F32, isOutput=False
    )
    thr_d = nc.declare_dram_parameter("thrcols", [128, 15], F32, isOutput=False)
    lut_d = nc.declare_dram_parameter("lutT", [C * K, M], BF16, isOutput=False)
    kvec_d = nc.declare_dram_parameter("kvec", [128, 1], F32, isOutput=False)
    out_d = nc.declare_dram_parameter("out", [N_CORE, M], BF16, isOutput=True)

    NH = ROWS_STAGE // 2  # 1024 rows per partition-half

    with tile.TileContext(nc) as tc:
        from contextlib import ExitStack
        es = ExitStack()
        pers = es.enter_context(tc.tile_pool(name="pers", bufs=1))

        def ptile(shape, dtype, name):
            return pers.tile(shape, dtype, name=name, tag=name)

        # ---- persistent tiles ----
        lutT = ptile([128, 8, M], BF16, "lutT_sb")       # [j*128+p row, m]
        thr = ptile([128, 15], F32, "thr_sb")
        kvec = ptile([128, 1], F32, "kvec_sb")
        tmps = [ptile([128, NH], F32, f"tmp{ti}_sb") for ti in range(10)]
        b0, b1, b2, tA, tB, tC, tD, tE, tF, tG = tmps
        I8 = mybir.dt.int8
        b0i = ptile([128, NH], I8, "b0i_sb")
        b1i = ptile([128, NH], I8, "b1i_sb")

        xpool = es.enter_context(tc.tile_pool(name="xpool", bufs=2))
        bpool = es.enter_context(tc.tile_pool(name="bpool", bufs=2))
        epool = es.enter_context(tc.tile_pool(name="epool", bufs=2))
        opool = es.enter_context(tc.tile_pool(name="opool", bufs=2))
        pspool = es.enter_context(
            tc.tile_pool(name="pspool", bufs=2, space=bass.MemorySpace.PSUM)
        )

        nc.sync.dma_start(thr[:], thr_d[:])
        nc.sync.dma_start(kvec[:], kvec_d[:])
        for j in range(8):
            nc.scalar.dma_start(lutT[:, j, :], lut_d[j * 128:(j + 1) * 128, :])

        def tcol(i):
            return thr[:, i:i + 1]

        def descent(chT, bucketbf, fsl):
            """tree descent for n-slice fsl; writes bucket values to bucketbf."""
            xd = [chT[:, d, fsl] for d in range(DEPTH)]
            t = [tt[:, fsl] for tt in tmps]
            b0_, b1_, b2_, tA_, tB_, tC_, tD_, tE_, tF_, tG_ = t
            b0i_, b1i_ = b0i[:, fsl], b1i[:, fsl]
            V = nc.vector
            V.tensor_scalar(b0_, xd[0], tcol(0), None, ALU.is_gt)
            V.tensor_copy(b0i_, b0_)
            V.tensor_scalar(tA_, b0_, tcol(2), tcol(1), ALU.mult, ALU.add)
            V.tensor_tensor(b1_, xd[1], tA_, ALU.is_gt)
            V.tensor_copy(b1i_, b1_)

            V.tensor_scalar(tB_, b1_, tcol(4), tcol(3), ALU.mult, ALU.add)
            V.tensor_scalar(tC_, b1_, tcol(6), tcol(5), ALU.mult, ALU.add)
            V.tensor_copy(tA_, tB_)
            V.copy_predicated(tA_, b0i_, tC_)
            V.tensor_tensor(b2_, xd[2], tA_, ALU.is_gt)

            V.tensor_scalar(tB_, b2_, tcol(8), tcol(7), ALU.mult, ALU.add)
            V.tensor_scalar(tC_, b2_, tcol(10), tcol(9), ALU.mult, ALU.add)
            V.tensor_scalar(tD_, b2_, tcol(12), tcol(11), ALU.mult, ALU.add)
            V.tensor_scalar(tE_, b2_, tcol(14), tcol(13), ALU.mult, ALU.add)
            V.tensor_copy(tF_, tB_)
            V.copy_predicated(tF_, b1i_, tC_)
            V.tensor_copy(tG_, tD_)
            V.copy_predicated(tG_, b1i_, tE_)
            V.tensor_copy(tA_, tF_)
            V.copy_predicated(tA_, b0i_, tG_)
            V.tensor_tensor(tD_, xd[3], tA_, ALU.is_gt)   # b3 -> tD_

            # bucket = ((b0*2+b1)*2+b2)*2+b3
            V.scalar_tensor_tensor(tB_, b0_, 2.0, b1_, ALU.mult, ALU.add)
            V.scalar_tensor_tensor(tC_, tB_, 2.0, b2_, ALU.mult, ALU.add)
            V.scalar_tensor_tensor(bucketbf[:, fsl], tC_, 2.0, tD_, ALU.mult, ALU.add)

        for rep in range(repeat):
            for s in range(N_STAGES):
                # gathered input, already laid out [p=(hp*64+c), d, n];
                # per-plane DMAs so level-0 compares start after plane 0 lands
                chT = xpool.tile([128, DEPTH, NH], F32, name="chT", tag="chT")
                for d in range(DEPTH):
                    nc.sync.dma_start(chT[:, d, :], xg_d[s, :, d, :])
                bucketbf = bpool.tile([128, NH], BF16, name="bucketbf", tag="bucketbf")
                ET = epool.tile([128, 8, ROWS_STAGE], BF16, name="ET", tag="ET")

                # chunk stage 0 finely so first matmuls start early
                first = rep == 0 and s == 0
                widths = list(widths0) if first else [NH]
                f0 = 0
                for width in widths:
                    fsl = slice(f0, f0 + width)
                    descent(chT, bucketbf, fsl)

                    # E^T: replicate bucket row of channel 8j+cl to partitions
                    # p = k*8 + cl (log2 doubling), then compare k = p//8
                    engs = [nc.sync, nc.scalar]
                    for j in range(8):
                        for hp in range(2):
                            esl = slice(hp * NH + f0, hp * NH + f0 + width)
                            engs[(2 * j + hp) % 2].dma_start(
                                ET[0:8, j, esl],
                                bucketbf[hp * 64 + 8 * j:hp * 64 + 8 * j + 8, fsl],
                            )
                    for dbl in range(4):
                        w = 8 << dbl
                        for hp in range(2):
                            esl = slice(hp * NH + f0, hp * NH + f0 + width)
                            (nc.scalar if (dbl + hp) % 2 == 0 else nc.sync).dma_start(
                                ET[w:2 * w, :, esl], ET[0:w, :, esl]
                            )
                    for j in range(8):
                        for hp in range(2):
                            esl = slice(hp * NH + f0, hp * NH + f0 + width)
                            nc.vector.tensor_scalar(
                                ET[:, j, esl], ET[:, j, esl], kvec[:], None,
                                ALU.is_equal,
                            )

                    # ---- matmul + output for the i-tiles this chunk covers ----
                    i_list = [hp * 8 + f0 // 128 + d
                              for hp in range(2) for d in range(width // 128)]
                    for i in i_list:
                        ps = [
                            pspool.tile([128, 512], F32, name=f"ps{mc}", tag=f"ps{mc}")
                            for mc in range(2)
                        ]
                        for j in range(8):
                            lhsT = ET[:, j, i * 128:(i + 1) * 128]
                            for mc in range(2):
                                nc.tensor.matmul(
                                    ps[mc][:], lhsT,
                                    lutT[:, j, mc * 512:(mc + 1) * 512],
                                    start=(j == 0), stop=(j == 7),
                                )
                        osb = opool.tile([128, M], BF16, name="osb", tag="osb")
                        nc.scalar.activation(osb[:, 0:512], ps[0][:], AFT.Copy)
                        nc.vector.tensor_copy(osb[:, 512:1024], ps[1][:])
                        r0 = s * ROWS_STAGE + i * 128
                        (nc.sync if i % 2 == 0 else nc.scalar).dma_start(
                            out_d[r0:r0 + 128, :], osb[:]
                        )
                    f0 += width
        es.close()
    nc.finalize()
    return nc


def _prep_inputs(inputMatrix, dims, thresholds, lut):
    x = np.asarray(inputMatrix, dtype=np.float32)
    dims_a = np.asarray(dims).ravel().astype(np.int64)
    thr = np.asarray(thresholds, dtype=np.float32).reshape(C, K - 1)
    lut = np.asarray(lut, dtype=np.float32)

    # thrcols [128, 15]: t0,t1,d21,t3,d43,t5,d65,t7,d87,t9,d109,t11,d1211,t13,d1413
    tcols = np.empty((C, 15), dtype=np.float32)
    tcols[:, 0] = thr[:, 0]
    pairs = [(1, 2), (3, 4), (5, 6), (7, 8), (9, 10), (11, 12), (13, 14)]
    for idx, (lo, hi) in enumerate(pairs):
        tcols[:, 1 + 2 * idx] = thr[:, lo]
        tcols[:, 2 + 2 * idx] = thr[:, hi] - thr[:, lo]
    thrcols = np.concatenate([tcols, tcols], axis=0)  # [128, 15]

    # lutT row p = j*128 + k*8 + cl  (channel = 8j+cl), col m
    lutT = (
        lut.reshape(M, 8, 8, K)           # [M, j, cl, k]
        .transpose(1, 3, 2, 0)            # [j, k, cl, M]
        .reshape(C * K, M)
        .astype(ml_dtypes.bfloat16)
    )

    kvec = (np.arange(128) // 8).astype(np.float32).reshape(128, 1)

    # per-core gathered input: xg[s][hp*64+c][d][n] =
    #   x[core*4096 + s*2048 + hp*1024 + n, dims[4c+d]]
    NH = ROWS_STAGE // 2
    xgs = []
    for i in range(N_CORES):
        g = x[i * N_CORE:(i + 1) * N_CORE][:, dims_a]        # [4096, 256]
        g = g.reshape(N_STAGES, 2, NH, C, DEPTH)             # [s, hp, n, c, d]
        xg = np.ascontiguousarray(g.transpose(0, 1, 3, 4, 2)).reshape(
            N_STAGES, 128, DEPTH, NH
        )
        xgs.append(xg)
    return xgs, thrcols, lutT, kvec


def make_in_maps(prepped):
    xgs, thrcols, lutT, kvec = prepped
    return [
        {"xg": xgs[i], "thrcols": thrcols, "lutT": lutT, "kvec": kvec}
        for i in range(N_CORES)
    ]


def kernel(inputMatrix, dims, thresholds, lut, selection_matrix=None,
           tree_des_mat=None):
    from concourse.bass_utils import run_bass_kernel_spmd

    in_maps = make_in_maps(_prep_inputs(inputMatrix, dims, thresholds, lut))
    nc = build_program()
    res = run_bass_kernel_spmd(nc, in_maps, list(range(N_CORES)))
    out = np.concatenate(
        [np.asarray(res.results[i]["out"]) for i in range(N_CORES)], axis=0
    )
    return out.astype(np.float32)


# revision 18
# speedup vs baseline: 4.7381x; 1.0789x over previous
import sys

if "/opt/trn_rl_repo" not in sys.path:
    sys.path.insert(0, "/opt/trn_rl_repo")

import numpy as np
import ml_dtypes

from concourse import bass, tile, bacc
from concourse.bass import mybir

F32 = mybir.dt.float32
BF16 = mybir.dt.bfloat16

N_CORES = 8
N_TOTAL = 32768
N_CORE = N_TOTAL // N_CORES  # 4096 rows per core
D = 1024
C = 64
K = 16
DEPTH = 4
M = 1024
N_STAGES = 2
ROWS_STAGE = N_CORE // N_STAGES  # 2048
ALU = mybir.AluOpType
AFT = mybir.ActivationFunctionType


def build_program(dims=None, repeat=1, widths0=(256, 256, 512), ps_bufs=2):
    """dims kept for signature compat (gather now happens on host)."""
    nc = bacc.Bacc()
    xg_d = nc.declare_dram_parameter(
        "xg", [N_STAGES, 128, DEPTH, ROWS_STAGE // 2], F32, isOutput=False
    )
    thr_d = nc.declare_dram_parameter("thrcols", [128, 15], F32, isOutput=False)
    lut_d = nc.declare_dram_parameter("lutT", [C * K, M], BF16, isOutput=False)
    kvec_d = nc.declare_dram_parameter("kvec", [128, 1], F32, isOutput=False)
    out_d = nc.declare_dram_parameter("out", [N_CORE, M], BF16, isOutput=True)

    NH = ROWS_STAGE // 2  # 1024 rows per partition-half

    with tile.TileContext(nc) as tc:
        from contextlib import ExitStack
        es = ExitStack()
        pers = es.enter_context(tc.tile_pool(name="pers", bufs=1))

        def ptile(shape, dtype, name):
            return pers.tile(shape, dtype, name=name, tag=name)

        # ---- persistent tiles ----
        lutT = ptile([128, 8, M], BF16, "lutT_sb")       # [j*128+p row, m]
        thr = ptile([128, 15], F32, "thr_sb")
        kvec = ptile([128, 1], F32, "kvec_sb")
        tmps = [ptile([128, NH], F32, f"tmp{ti}_sb") for ti in range(10)]
        b0, b1, b2, tA, tB, tC, tD, tE, tF, tG = tmps
        I8 = mybir.dt.int8
        b0i = ptile([128, NH], I8, "b0i_sb")
        b1i = ptile([128, NH], I8, "b1i_sb")

        xpool = es.enter_context(tc.tile_pool(name="xpool", bufs=2))
        bpool = es.enter_context(tc.tile_pool(name="bpool", bufs=2))
        epool = es.enter_context(tc.tile_pool(name="epool", bufs=2))
        opool = es.enter_context(tc.tile_pool(name="opool", bufs=2))
        pspool = es.enter_context(
            tc.tile_pool(name="pspool", bufs=ps_bufs, space=bass.MemorySpace.PSUM)
        )

        nc.sync.dma_start(thr[:], thr_d[:])
        nc.sync.dma_start(kvec[:], kvec_d[:])
        for j in range(8):
            nc.scalar.dma_start(lutT[:, j, :], lut_d[j * 128:(j + 1) * 128, :])

        def tcol(i):
            return thr[:, i:i + 1]

        def descent(chT, bucketbf, fsl):
            """tree descent for n-slice fsl; writes bucket values to bucketbf."""
            xd = [chT[:, d, fsl] for d in range(DEPTH)]
            t = [tt[:, fsl] for tt in tmps]
            b0_, b1_, b2_, tA_, tB_, tC_, tD_, tE_, tF_, tG_ = t
            b0i_, b1i_ = b0i[:, fsl], b1i[:, fsl]
            V = nc.vector
            V.tensor_scalar(b0_, xd[0], tcol(0), None, ALU.is_gt)
            V.tensor_copy(b0i_, b0_)
            V.tensor_scalar(tA_, b0_, tcol(2), tcol(1), ALU.mult, ALU.add)
            V.tensor_tensor(b1_, xd[1], tA_, ALU.is_gt)
            V.tensor_copy(b1i_, b1_)

            V.tensor_scalar(tB_, b1_, tcol(4), tcol(3), ALU.mult, ALU.add)
            V.tensor_scalar(tC_, b1_, tcol(6), tcol(5), ALU.mult, ALU.add)
            V.tensor_copy(tA_, tB_)
            V.copy_predicated(tA_, b0i_, tC_)
            V.tensor_tensor(b2_, xd[2], tA_, ALU.is_gt)

            V.tensor_scalar(tB_, b2_, tcol(8), tcol(7), ALU.mult, ALU.add)
            V.tensor_scalar(tC_, b2_, tcol(10), tcol(9), ALU.mult, ALU.add)
            V.tensor_scalar(tD_, b2_, tcol(12), tcol(11), ALU.mult, ALU.add)
            V.tensor_scalar(tE_, b2_, tcol(14), tcol(13), ALU.mult, ALU.add)
            V.tensor_copy(tF_, tB_)
            V.copy_predicated(tF_, b1i_, tC_)
            V.tensor_copy(tG_, tD_)
            V.copy_predicated(tG_, b1i_, tE_)
            V.tensor_copy(tA_, tF_)
            V.copy_predicated(tA_, b0i_, tG_)
            V.tensor_tensor(tD_, xd[3], tA_, ALU.is_gt)   # b3 -> tD_

            # bucket = ((b0*2+b1)*2+b2)*2+b3
            V.scalar_tensor_tensor(tB_, b0_, 2.0, b1_, ALU.mult, ALU.add)
            V.scalar_tensor_tensor(tC_, tB_, 2.0, b2_, ALU.mult, ALU.add)
            V.scalar_tensor_tensor(bucketbf[:, fsl], tC_, 2.0, tD_, ALU.mult, ALU.add)

        for rep in range(repeat):
            for s in range(N_STAGES):
                # gathered input, already laid out [p=(hp*64+c), d, n];
                # per-plane DMAs so level-0 compares start after plane 0 lands
                chT = xpool.tile([128, DEPTH, NH], F32, name="chT", tag="chT")
                for d in range(DEPTH):
                    nc.sync.dma_start(chT[:, d, :], xg_d[s, :, d, :])
                bucketbf = bpool.tile([128, NH], BF16, name="bucketbf", tag="bucketbf")
                ET = epool.tile([128, 8, ROWS_STAGE], BF16, name="ET", tag="ET")

                # chunk stage 0 finely so first matmuls start early
                first = rep == 0 and s == 0
                widths = list(widths0) if first else [NH]
                f0 = 0
                for width in widths:
                    fsl = slice(f0, f0 + width)
                    descent(chT, bucketbf, fsl)

                    # E^T: replicate bucket row of channel 8j+cl to partitions
                    # p = k*8 + cl (log2 doubling), then compare k = p//8
                    engs = [nc.sync, nc.scalar]
                    for j in range(8):
                        for hp in range(2):
                            esl = slice(hp * NH + f0, hp * NH + f0 + width)
                            engs[(2 * j + hp) % 2].dma_start(
                                ET[0:8, j, esl],
                                bucketbf[hp * 64 + 8 * j:hp * 64 + 8 * j + 8, fsl],
                            )
                    for dbl in range(4):
                        w = 8 << dbl
                        for hp in range(2):
                            esl = slice(hp * NH + f0, hp * NH + f0 + width)
                            (nc.scalar if (dbl + hp) % 2 == 0 else nc.sync).dma_start(
                                ET[w:2 * w, :, esl], ET[0:w, :, esl]
                            )
                    if width == NH:
                        for j in range(8):
                            nc.vector.tensor_scalar(
                                ET[:, j, :], ET[:, j, :], kvec[:], None,
                                ALU.is_equal,
                            )
                    else:
                        for j in range(8):
                            for hp in range(2):
                                esl = slice(hp * NH + f0, hp * NH + f0 + width)
                                nc.vector.tensor_scalar(
                                    ET[:, j, esl], ET[:, j, esl], kvec[:], None,
                                    ALU.is_equal,
                                )

                    # ---- matmul + output for the i-tiles this chunk covers ----
                    i_list = [hp * 8 + f0 // 128 + d
                              for hp in range(2) for d in range(width // 128)]
                    for i in i_list:
                        ps = [
                            pspool.tile([128, 512], F32, name=f"ps{mc}", tag=f"ps{mc}")
                            for mc in range(2)
                        ]
                        for j in range(8):
                            lhsT = ET[:, j, i * 128:(i + 1) * 128]
                            for mc in range(2):
                                nc.tensor.matmul(
                                    ps[mc][:], lhsT,
                                    lutT[:, j, mc * 512:(mc + 1) * 512],
                                    start=(j == 0), stop=(j == 7),
                                )
                        osb = opool.tile([128, M], BF16, name="osb", tag="osb")
                        nc.scalar.activation(osb[:, 0:512], ps[0][:], AFT.Copy)
                        nc.vector.tensor_copy(osb[:, 512:1024], ps[1][:])
                        r0 = s * ROWS_STAGE + i * 128
                        (nc.sync if i % 2 == 0 else nc.scalar).dma_start(
                            out_d[r0:r0 + 128, :], osb[:]
                        )
                    f0 += width
        es.close()
    nc.finalize()
    return nc


def _prep_inputs(inputMatrix, dims, thresholds, lut):
    x = np.asarray(inputMatrix, dtype=np.float32)
    dims_a = np.asarray(dims).ravel().astype(np.int64)
    thr = np.asarray(thresholds, dtype=np.float32).reshape(C, K - 1)
    lut = np.asarray(lut, dtype=np.float32)

    # thrcols [128, 15]: t0,t1,d21,t3,d43,t5,d65,t7,d87,t9,d109,t11,d1211,t13,d1413
    tcols = np.empty((C, 15), dtype=np.float32)
    tcols[:, 0] = thr[:, 0]
    pairs = [(1, 2), (3, 4), (5, 6), (7, 8), (9, 10), (11, 12), (13, 14)]
    for idx, (lo, hi) in enumerate(pairs):
        tcols[:, 1 + 2 * idx] = thr[:, lo]
        tcols[:, 2 + 2 * idx] = thr[:, hi] - thr[:, lo]
    thrcols = np.concatenate([tcols, tcols], axis=0)  # [128, 15]

    # lutT row p = j*128 + k*8 + cl  (channel = 8j+cl), col m
    lutT = (
        lut.reshape(M, 8, 8, K)           # [M, j, cl, k]
        .transpose(1, 3, 2, 0)            # [j, k, cl, M]
        .reshape(C * K, M)
        .astype(ml_dtypes.bfloat16)
    )

    kvec = (np.arange(128) // 8).astype(np.float32).reshape(128, 1)

    # per-core gathered input: xg[s][hp*64+c][d][n] =
    #   x[core*4096 + s*2048 + hp*1024 + n, dims[4c+d]]
    NH = ROWS_STAGE // 2
    xgs = []
    for i in range(N_CORES):
        g = x[i * N_CORE:(i + 1) * N_CORE][:, dims_a]        # [4096, 256]
        g = g.reshape(N_STAGES, 2, NH, C, DEPTH)             # [s, hp, n, c, d]
        xg = np.ascontiguousarray(g.transpose(0, 1, 3, 4, 2)).reshape(
            N_STAGES, 128, DEPTH, NH
        )
        xgs.append(xg)
    return xgs, thrcols, lutT, kvec


def make_in_maps(prepped):
    xgs, thrcols, lutT, kvec = prepped
    return [
        {"xg": xgs[i], "thrcols": thrcols, "lutT": lutT, "kvec": kvec}
        for i in range(N_CORES)
    ]


def kernel(inputMatrix, dims, thresholds, lut, selection_matrix=None,
           tree_des_mat=None):
    from concourse.bass_utils import run_bass_kernel_spmd

    in_maps = make_in_maps(_prep_inputs(inputMatrix, dims, thresholds, lut))
    nc = build_program()
    res = run_bass_kernel_spmd(nc, in_maps, list(range(N_CORES)))
    out = np.concatenate(
        [np.asarray(res.results[i]["out"]) for i in range(N_CORES)], axis=0
    )
    return out.astype(np.float32)
